# revision 20
# baseline (speedup 1.0000x reference)
"""Expert-parallel MoE (Mixtral-style top-2 of 8 experts, SwiGLU) on 8 TRN2 cores.

Two-slot layout: every core hosts HALF (along the intermediate dim) of two
experts — one "heavy" expert (slot 0, column capacity C_a) and one "light"
expert (slot 1, C_b).  Each expert's I=4096 channels are split across two
cores; the two half-results are summed on host.  This balances PE work across
cores: streaming cycles drop from 1536*C_max to 768*(C_a+C_b).

Routing (softmax/top-k/renorm, 1024x8) runs on host during input sharding;
the routing weight is folded in on host during the combine.

Device kernel per core, per slot (all matmuls bf16, fp32 PSUM accumulation):
  phase B:  G[i,t] = W13T[h,i].T-accum over h of x[h,t]  (gate|up pairs)
            act[i,t] = sigmoid(G_g)*G_g*G_u              (bf16 in SBUF)
  phase D:  y[hh,t] = sum_i act[i,t].T @ W2T[i,hh]       (j-major waves of
            <=6 PSUM accumulators so W2 tiles are consumed in DMA order)

DMA notes: every DMA instruction costs ~0.6us of sequencer issue time, so
weights stream on the Sync ring while xt/y ride the Scalar (ACT) ring, and
the first tiles are split into small chunks so the PE can start ~8us in.
"""

import os

import ml_dtypes
import numpy as np

import concourse.bass as bass
from concourse import bacc
import concourse.mybir as mybir
import concourse.tile as tile
from concourse.bass_utils import run_bass_kernel_spmd

P = 128
H = 2048            # hidden dim
INTER = 4096        # intermediate dim
E = 8               # experts
N_CORES = 8
N_SLOTS = 2         # experts hosted per core (half-I each)
IH = INTER // N_SLOTS   # 2048 intermediate channels per slot
KO = H // P         # 16 contraction steps over hidden dim
NJ = IH // P        # 16 i-tiles per slot
HC = H // P         # 16 output row tiles
WAVES = [(0, 6), (6, 6), (12, 4)]   # down-proj hh waves (PSUM: 6+2 banks max)
# first-load chunking (ko ranges) for slot 0 so the PE can start early
HEAD_CHUNKS = [(0, 1), (1, 3), (4, 4), (8, 8)]
BF16 = mybir.dt.bfloat16
F32 = mybir.dt.float32

# set by kernel() for test harness introspection
last_results = None


def _build_nc(caps: list[int]) -> bass.Bass:
    act_fn = mybir.ActivationFunctionType
    CMAX = max(caps)

    nc = bacc.Bacc()
    xt_d, w13_d, w2_d, y_d = [], [], [], []
    for s, C in enumerate(caps):
        xt_d.append(nc.declare_dram_parameter(f"xt{s}", [P, KO, C], BF16, isOutput=False))
        w13_d.append(nc.declare_dram_parameter(f"w13{s}", [NJ, P, KO, 2 * P], BF16, isOutput=False))
        w2_d.append(nc.declare_dram_parameter(f"w2{s}", [NJ, P, H], BF16, isOutput=False))
        y_d.append(nc.declare_dram_parameter(f"y{s}", [H, C], BF16, isOutput=True))

    with tile.TileContext(nc) as tc:
        with (
            tc.tile_pool(name="xp", bufs=1) as xp,
            tc.tile_pool(name="w13p", bufs=10) as w13p,
            tc.tile_pool(name="w13hp", bufs=1) as w13hp,
            tc.tile_pool(name="w2p", bufs=1) as w2p,
            tc.tile_pool(name="actp", bufs=1) as actp,
            tc.tile_pool(name="silup", bufs=2) as silup,
            tc.tile_pool(name="outp", bufs=2) as outp,
            tc.tile_pool(name="psB", bufs=1, space="PSUM") as psB,
            tc.tile_pool(name="psD", bufs=1, space="PSUM") as psD,
        ):
            # HAM warm-up: ~4us of dummy back-to-back matmuls on a memset
            # scratch tile so the PE clock-gate opens (1.2 -> 2.4 GHz) before
            # the first real matmul; the real j=0..2 then run at full rate.
            scratch = xp.tile([P, CMAX], BF16, tag="warm_sb", name="warm_sb")
            nc.gpsimd.memset(scratch[:], 0)
            warm_ps = psD.tile([P, CMAX], F32, tag="acc_0", name="warm_ps")
            N_WARM = 27  # ~6.8us cold: spans two full HAM windows so one is busy
            for i in range(N_WARM):
                nc.tensor.matmul(
                    warm_ps[:],
                    scratch[:, :P],
                    scratch[:],
                    start=(i == 0),
                    stop=(i == N_WARM - 1),
                )

            # xt in ramped chunks on the ACT ring; weights own the SP ring.
            # Chunk tags are shared across slots (bufs=2): the next slot's
            # DMAs wait for this slot's last B-phase read, landing during
            # this slot's down phase.
            def load_xt(s):
                tiles = [None] * KO
                for ci, (k0, kn) in enumerate(HEAD_CHUNKS):
                    t = xp.tile([P, kn, CMAX], BF16, tag=f"xc_{ci}", bufs=2, name=f"xc{s}_{ci}")
                    nc.scalar.dma_start(t[:, :, : caps[s]], xt_d[s][:, k0 : k0 + kn, :])
                    for r in range(kn):
                        tiles[k0 + r] = t[:, r, :]
                return tiles

            xt_all = [load_xt(0)] + [None] * (N_SLOTS - 1)

            # slot-0 j=0 weights in the same ramped chunks (SP ring)
            heads = []
            for ci, (k0, kn) in enumerate(HEAD_CHUNKS):
                ht = w13hp.tile([P, kn, 2 * P], BF16, tag=f"h_{ci}", name=f"w13h{ci}")
                nc.sync.dma_start(ht[:], w13_d[0][0][:, k0 : k0 + kn, :])
                heads.append(ht)

            def head_slice(ko, half):
                for ci, (k0, kn) in enumerate(HEAD_CHUNKS):
                    if k0 <= ko < k0 + kn:
                        return heads[ci][:, ko - k0, half * P : (half + 1) * P]
                raise AssertionError

            for s, C in enumerate(caps):
                xt_tiles = xt_all[s]
                act_tiles = []
                w2_tiles = []
                # ---- phase B: gate/up projections + SwiGLU ----
                for j in range(NJ):
                    if s == 0 and j == 0:
                        w13_sl = head_slice
                    elif s == 0 and j <= 8:
                        # DMA ramp-up region: half-tile transfers so the PE
                        # trickles along instead of hitting one long stall
                        ha = w13p.tile([P, KO // 2, 2 * P], BF16, tag="w13e", bufs=4, name=f"w13_{s}_{j}a")
                        hb = w13p.tile([P, KO // 2, 2 * P], BF16, tag="w13e", bufs=4, name=f"w13_{s}_{j}b")
                        nc.sync.dma_start(ha[:], w13_d[s][j][:, : KO // 2, :])
                        nc.sync.dma_start(hb[:], w13_d[s][j][:, KO // 2 :, :])

                        def w13_sl(ko, half, _a=ha, _b=hb):
                            t = _a if ko < KO // 2 else _b
                            return t[:, ko % (KO // 2), half * P : (half + 1) * P]
                    else:
                        w13_sb = w13p.tile([P, KO, 2 * P], BF16, tag="w13", bufs=8, name=f"w13_{s}_{j}")
                        nc.sync.dma_start(w13_sb[:], w13_d[s][j])

                        def w13_sl(ko, half, _t=w13_sb):
                            return _t[:, ko, half * P : (half + 1) * P]

                    g_ps = psB.tile([P, CMAX], F32, tag="g", name=f"g_{s}_{j}")
                    u_ps = psB.tile([P, CMAX], F32, tag="u", name=f"u_{s}_{j}")
                    for half, ps in ((0, g_ps), (1, u_ps)):
                        for ko in range(KO):
                            nc.tensor.matmul(
                                ps[:, :C],
                                w13_sl(ko, half),
                                xt_tiles[ko][:, :C],
                                start=(ko == 0),
                                stop=(ko == KO - 1),
                            )
                    # silu: s*g first so the g PSUM bank frees before gate j+1
                    s_sb = silup.tile([P, CMAX], F32, tag="sig", name=f"sig_{s}_{j}")
                    nc.scalar.activation(s_sb[:, :C], g_ps[:, :C], act_fn.Sigmoid)
                    sg_sb = silup.tile([P, CMAX], F32, tag="sg", name=f"sg_{s}_{j}")
                    nc.vector.tensor_mul(sg_sb[:, :C], s_sb[:, :C], g_ps[:, :C])
                    a_sb = actp.tile([P, CMAX], BF16, tag=f"act_{j}", name=f"act_{s}_{j}")
                    nc.vector.tensor_mul(a_sb[:, :C], sg_sb[:, :C], u_ps[:, :C])
                    act_tiles.append(a_sb)

                # next slot's xt: emitted after the last sigmoid so the wait
                # (on this slot's final B-phase reads) can't head-of-line
                # block the ACT ring; lands during this slot's down phase
                if s + 1 < N_SLOTS:
                    xt_all[s + 1] = load_xt(s + 1)

                # down-proj weights queue on the SP ring behind this slot's w13s
                for j in range(NJ):
                    w2_sb = w2p.tile([P, H], BF16, tag=f"w2_{j}", name=f"w2_{s}_{j}")
                    nc.sync.dma_start(w2_sb[:], w2_d[s][j])
                    w2_tiles.append(w2_sb)

                # ---- phase D: down-proj over hh ----
                # first JWAVE hh run j-major so W2 tiles are consumed in DMA
                # arrival order; the rest run hh-major (W2 resident by then)
                # so each hh's cast+writeback staggers under the next hh's
                # matmuls.  Writeback DMAs ride the SP ring: an issue on the
                # ACT ring would head-of-line-block the next slot's sigmoids.
                def writeback(acc, hh, split=False):
                    o_sb = outp.tile([P, CMAX], BF16, tag="o", name=f"o_{s}_{hh}")
                    if split:
                        # very last writeback: halve it so the second DMA's
                        # issue overlaps the first cast instead of trailing
                        ch = (C // 2 + 15) // 16 * 16
                        for c0, c1 in ((0, ch), (ch, C)):
                            nc.vector.tensor_copy(o_sb[:, c0:c1], acc[:, c0:c1])
                            nc.sync.dma_start(y_d[s][hh * P : (hh + 1) * P, c0:c1], o_sb[:, c0:c1])
                    else:
                        nc.vector.tensor_copy(o_sb[:, :C], acc[:, :C])
                        nc.sync.dma_start(y_d[s][hh * P : (hh + 1) * P, :], o_sb[:, :C])

                JWAVE = 6
                accs = [
                    psD.tile([P, CMAX], F32, tag=f"acc_{k}", name=f"acc_{s}_{k}")
                    for k in range(JWAVE)
                ]
                for j in range(NJ):
                    for k in range(JWAVE):
                        nc.tensor.matmul(
                            accs[k][:, :C],
                            w2_tiles[j][:, k * P : (k + 1) * P],
                            act_tiles[j][:, :C],
                            start=(j == 0),
                            stop=(j == NJ - 1),
                        )
                        if j == NJ - 1:
                            writeback(accs[k], k)
                for hh in range(JWAVE, HC):
                    acc = psD.tile([P, CMAX], F32, tag=f"acc_{hh % JWAVE}", name=f"acc_{s}_{hh}")
                    for j in range(NJ):
                        nc.tensor.matmul(
                            acc[:, :C],
                            w2_tiles[j][:, hh * P : (hh + 1) * P],
                            act_tiles[j][:, :C],
                            start=(j == 0),
                            stop=(j == NJ - 1),
                        )
                    writeback(acc, hh, split=(s == N_SLOTS - 1 and hh == HC - 1))
    nc.compile()
    return nc


def _route(router_logits: np.ndarray, top_k: int):
    """Match jax.nn.softmax + jax.lax.top_k + renormalize (ties -> lower idx)."""
    p = router_logits.astype(np.float64)
    p = np.exp(p - p.max(axis=-1, keepdims=True))
    p /= p.sum(axis=-1, keepdims=True)
    order = np.argsort(-p, axis=-1, kind="stable")
    idx = order[:, :top_k]
    w = np.take_along_axis(p, idx, axis=-1)
    w /= w.sum(axis=-1, keepdims=True)
    return idx, w


def kernel(hidden_states, router_logits, W13, W2, top_k):
    global last_results
    top_k = int(top_k)
    hs = np.asarray(hidden_states, dtype=np.float32)
    T = hs.shape[0]
    idx, w = _route(np.asarray(router_logits, dtype=np.float32), top_k)

    tok_ids, tok_w = [], []
    for e in range(E):
        sel = idx == e  # [T, k]; at most one True per row
        rows = np.nonzero(sel.any(axis=-1))[0]
        tok_ids.append(rows)
        tok_w.append(w[sel].astype(np.float32))  # row-major -> token order

    counts = np.array([len(r) for r in tok_ids])
    order = np.argsort(-counts, kind="stable")
    groups = [order[:4], order[4:]]  # heavy experts in slot 0, light in slot 1

    def pad16(n):
        return max(16, -(-n // 16) * 16)

    caps = [pad16(int(counts[g].max())) for g in groups]
    assert caps[0] <= 512, "column capacity exceeds one PSUM bank"

    W13 = np.asarray(W13, dtype=np.float32)
    W2 = np.asarray(W2, dtype=np.float32)

    in_maps = [dict() for _ in range(N_CORES)]
    for c in range(N_CORES):
        for s in range(N_SLOTS):
            e = int(groups[s][c // 2])
            h = c % 2  # which half of the expert's I channels
            C = caps[s]
            rows = tok_ids[e]
            n = len(rows)
            xt = np.zeros((P, KO, C), dtype=ml_dtypes.bfloat16)
            if n:
                xg = hs[rows].astype(ml_dtypes.bfloat16)  # [n, H]
                xt[:, :, :n] = xg.T.reshape(KO, P, n).transpose(1, 0, 2)
            gate = W13[e][h * IH : (h + 1) * IH]                    # [IH, H]
            up = W13[e][INTER + h * IH : INTER + (h + 1) * IH]     # [IH, H]
            blk = np.concatenate(
                [gate.reshape(NJ, P, H), up.reshape(NJ, P, H)], axis=1
            )  # [NJ, 2P(i), H]
            w13 = np.ascontiguousarray(
                blk.reshape(NJ, 2 * P, KO, P).transpose(0, 3, 2, 1)
            ).astype(ml_dtypes.bfloat16)  # [NJ, P(h), KO, 2P(i)]
            cols = W2[e][:, h * IH : (h + 1) * IH]  # [H, IH]
            w2 = np.ascontiguousarray(cols.T.reshape(NJ, P, H)).astype(
                ml_dtypes.bfloat16
            )  # [NJ, P(i), H]
            in_maps[c][f"xt{s}"] = xt
            in_maps[c][f"w13{s}"] = w13
            in_maps[c][f"w2{s}"] = w2

    nc = _build_nc(caps)
    res = run_bass_kernel_spmd(
        nc,
        in_maps,
        list(range(N_CORES)),
        trace=bool(os.environ.get("MOE_TRACE")),
        tmpdir=os.environ.get("MOE_TRACE_DIR") or None,
    )
    last_results = res

    out = np.zeros((T, H), dtype=np.float32)
    for s in range(N_SLOTS):
        for k in range(len(groups[s])):
            e = int(groups[s][k])
            rows = tok_ids[e]
            n = len(rows)
            if not n:
                continue
            y0 = np.asarray(res.results[2 * k][f"y{s}"], dtype=np.float32)
            y1 = np.asarray(res.results[2 * k + 1][f"y{s}"], dtype=np.float32)
            out[rows] += (y0[:, :n] + y1[:, :n]).T * tok_w[e][:, None]
    return out


# revision 22
# speedup vs baseline: 1.0589x; 1.0589x over previous
"""Expert-parallel MoE (Mixtral-style top-2 of 8 experts, SwiGLU) on 8 TRN2 cores.

Two-slot layout: every core hosts HALF (along the intermediate dim) of two
experts — one "heavy" expert (slot 0, column capacity C_a) and one "light"
expert (slot 1, C_b).  Each expert's I=4096 channels are split across two
cores; the two half-results are summed on host.  This balances PE work across
cores: streaming cycles drop from 1536*C_max to 768*(C_a+C_b).

Routing (softmax/top-k/renorm, 1024x8) runs on host during input sharding;
the routing weight is folded in on host during the combine.

Device kernel per core, per slot (all matmuls bf16, fp32 PSUM accumulation):
  phase B:  G[i,t] = W13T[h,i].T-accum over h of x[h,t]  (gate|up pairs)
            act[i,t] = sigmoid(G_g)*G_g*G_u              (bf16 in SBUF)
  phase D:  y[hh,t] = sum_i act[i,t].T @ W2T[i,hh]       (j-major waves of
            <=6 PSUM accumulators so W2 tiles are consumed in DMA order)

DMA notes: every DMA instruction costs ~0.6us of sequencer issue time, so
weights stream on the Sync ring while xt/y ride the Scalar (ACT) ring, and
the first tiles are split into small chunks so the PE can start ~8us in.
"""

import os

import ml_dtypes
import numpy as np

import concourse.bass as bass
from concourse import bacc
import concourse.mybir as mybir
import concourse.tile as tile
from concourse.bass_utils import run_bass_kernel_spmd

P = 128
H = 2048            # hidden dim
INTER = 4096        # intermediate dim
E = 8               # experts
N_CORES = 8
N_SLOTS = 2         # experts hosted per core (half-I each)
IH = INTER // N_SLOTS   # 2048 intermediate channels per slot
KO = H // P         # 16 contraction steps over hidden dim
NJ = IH // P        # 16 i-tiles per slot
HC = H // P         # 16 output row tiles
WAVES = [(0, 6), (6, 6), (12, 4)]   # down-proj hh waves (PSUM: 6+2 banks max)
# first-load chunking (ko ranges) for slot 0 so the PE can start early
HEAD_CHUNKS = [(0, 1), (1, 3), (4, 4), (8, 8)]
BF16 = mybir.dt.bfloat16
F32 = mybir.dt.float32

# set by kernel() for test harness introspection
last_results = None


def _build_nc(caps: list[int]) -> bass.Bass:
    act_fn = mybir.ActivationFunctionType
    CMAX = max(caps)

    nc = bacc.Bacc()
    xt_d, w13_d, w2_d, y_d = [], [], [], []
    for s, C in enumerate(caps):
        xt_d.append(nc.declare_dram_parameter(f"xt{s}", [P, KO, C], BF16, isOutput=False))
        w13_d.append(nc.declare_dram_parameter(f"w13{s}", [NJ, P, KO, 2 * P], BF16, isOutput=False))
        w2_d.append(nc.declare_dram_parameter(f"w2{s}", [NJ, P, H], BF16, isOutput=False))
        y_d.append(nc.declare_dram_parameter(f"y{s}", [H, C], BF16, isOutput=True))

    with tile.TileContext(nc) as tc:
        with (
            tc.tile_pool(name="xp", bufs=1) as xp,
            tc.tile_pool(name="w13p", bufs=10) as w13p,
            tc.tile_pool(name="w13hp", bufs=1) as w13hp,
            tc.tile_pool(name="w2p", bufs=1) as w2p,
            tc.tile_pool(name="actp", bufs=1) as actp,
            tc.tile_pool(name="silup", bufs=2) as silup,
            tc.tile_pool(name="outp", bufs=3) as outp,
            tc.tile_pool(name="psB", bufs=1, space="PSUM") as psB,
            tc.tile_pool(name="psD", bufs=1, space="PSUM") as psD,
        ):
            # HAM warm-up: ~4us of dummy back-to-back matmuls on a memset
            # scratch tile so the PE clock-gate opens (1.2 -> 2.4 GHz) before
            # the first real matmul; the real j=0..2 then run at full rate.
            scratch = xp.tile([P, CMAX], BF16, tag="warm_sb", name="warm_sb")
            nc.gpsimd.memset(scratch[:], 0)
            warm_ps = psD.tile([P, CMAX], F32, tag="acc_0", name="warm_ps")
            N_WARM = 27  # ~6.8us cold: spans two full HAM windows so one is busy
            for i in range(N_WARM):
                nc.tensor.matmul(
                    warm_ps[:],
                    scratch[:, :P],
                    scratch[:],
                    start=(i == 0),
                    stop=(i == N_WARM - 1),
                )

            # xt in ramped chunks on the ACT ring; weights own the SP ring.
            # Chunk tags are shared across slots (bufs=2): the next slot's
            # DMAs wait for this slot's last B-phase read, landing during
            # this slot's down phase.
            def load_xt(s):
                tiles = [None] * KO
                for ci, (k0, kn) in enumerate(HEAD_CHUNKS):
                    t = xp.tile([P, kn, CMAX], BF16, tag=f"xc_{ci}", bufs=2, name=f"xc{s}_{ci}")
                    nc.scalar.dma_start(t[:, :, : caps[s]], xt_d[s][:, k0 : k0 + kn, :])
                    for r in range(kn):
                        tiles[k0 + r] = t[:, r, :]
                return tiles

            xt_all = [load_xt(0)] + [None] * (N_SLOTS - 1)

            # slot-0 j=0 weights in the same ramped chunks (SP ring)
            heads = []
            for ci, (k0, kn) in enumerate(HEAD_CHUNKS):
                ht = w13hp.tile([P, kn, 2 * P], BF16, tag=f"h_{ci}", name=f"w13h{ci}")
                nc.sync.dma_start(ht[:], w13_d[0][0][:, k0 : k0 + kn, :])
                heads.append(ht)

            def head_slice(ko, half):
                for ci, (k0, kn) in enumerate(HEAD_CHUNKS):
                    if k0 <= ko < k0 + kn:
                        return heads[ci][:, ko - k0, half * P : (half + 1) * P]
                raise AssertionError

            for s, C in enumerate(caps):
                xt_tiles = xt_all[s]
                act_tiles = []
                w2_tiles = []
                # ---- phase B: gate/up projections + SwiGLU ----
                for j in range(NJ):
                    if s == 0 and j == 0:
                        w13_sl = head_slice
                    elif s == 0 and j <= 4:
                        # DMA ramp-up region: half-tile transfers so the PE
                        # trickles along instead of hitting one long stall
                        ha = w13p.tile([P, KO // 2, 2 * P], BF16, tag="w13e", bufs=4, name=f"w13_{s}_{j}a")
                        hb = w13p.tile([P, KO // 2, 2 * P], BF16, tag="w13e", bufs=4, name=f"w13_{s}_{j}b")
                        nc.sync.dma_start(ha[:], w13_d[s][j][:, : KO // 2, :])
                        nc.sync.dma_start(hb[:], w13_d[s][j][:, KO // 2 :, :])

                        def w13_sl(ko, half, _a=ha, _b=hb):
                            t = _a if ko < KO // 2 else _b
                            return t[:, ko % (KO // 2), half * P : (half + 1) * P]
                    else:
                        w13_sb = w13p.tile([P, KO, 2 * P], BF16, tag="w13", bufs=8, name=f"w13_{s}_{j}")
                        nc.sync.dma_start(w13_sb[:], w13_d[s][j])

                        def w13_sl(ko, half, _t=w13_sb):
                            return _t[:, ko, half * P : (half + 1) * P]

                    g_ps = psB.tile([P, CMAX], F32, tag="g", name=f"g_{s}_{j}")
                    u_ps = psB.tile([P, CMAX], F32, tag="u", name=f"u_{s}_{j}")
                    for half, ps in ((0, g_ps), (1, u_ps)):
                        for ko in range(KO):
                            nc.tensor.matmul(
                                ps[:, :C],
                                w13_sl(ko, half),
                                xt_tiles[ko][:, :C],
                                start=(ko == 0),
                                stop=(ko == KO - 1),
                            )
                    # silu: s*g first so the g PSUM bank frees before gate j+1
                    s_sb = silup.tile([P, CMAX], F32, tag="sig", name=f"sig_{s}_{j}")
                    nc.scalar.activation(s_sb[:, :C], g_ps[:, :C], act_fn.Sigmoid)
                    sg_sb = silup.tile([P, CMAX], F32, tag="sg", name=f"sg_{s}_{j}")
                    nc.vector.tensor_mul(sg_sb[:, :C], s_sb[:, :C], g_ps[:, :C])
                    a_sb = actp.tile([P, CMAX], BF16, tag=f"act_{j}", name=f"act_{s}_{j}")
                    nc.vector.tensor_mul(a_sb[:, :C], sg_sb[:, :C], u_ps[:, :C])
                    act_tiles.append(a_sb)

                # next slot's xt: emitted after the last sigmoid so the wait
                # (on this slot's final B-phase reads) can't head-of-line
                # block the ACT ring; lands during this slot's down phase
                if s + 1 < N_SLOTS:
                    xt_all[s + 1] = load_xt(s + 1)

                # down-proj weights queue on the SP ring behind this slot's w13s
                for j in range(NJ):
                    w2_sb = w2p.tile([P, H], BF16, tag=f"w2_{j}", name=f"w2_{s}_{j}")
                    nc.sync.dma_start(w2_sb[:], w2_d[s][j])
                    w2_tiles.append(w2_sb)

                # ---- phase D: down-proj over hh ----
                # first JWAVE hh run j-major so W2 tiles are consumed in DMA
                # arrival order; the rest run hh-major (W2 resident by then)
                # so each hh's cast+writeback staggers under the next hh's
                # matmuls.  Writeback DMAs ride the SP ring: an issue on the
                # ACT ring would head-of-line-block the next slot's sigmoids.
                def writeback(acc, hh, split=False):
                    o_sb = outp.tile([P, CMAX], BF16, tag="o", name=f"o_{s}_{hh}")
                    if split:
                        # very last writeback: halve it so the second DMA's
                        # issue overlaps the first cast instead of trailing
                        ch = (C // 2 + 15) // 16 * 16
                        for c0, c1 in ((0, ch), (ch, C)):
                            nc.vector.tensor_copy(o_sb[:, c0:c1], acc[:, c0:c1])
                            nc.sync.dma_start(y_d[s][hh * P : (hh + 1) * P, c0:c1], o_sb[:, c0:c1])
                    else:
                        nc.vector.tensor_copy(o_sb[:, :C], acc[:, :C])
                        nc.sync.dma_start(y_d[s][hh * P : (hh + 1) * P, :], o_sb[:, :C])

                JWAVE = 6
                accs = [
                    psD.tile([P, CMAX], F32, tag=f"acc_{k}", name=f"acc_{s}_{k}")
                    for k in range(JWAVE)
                ]
                for j in range(NJ):
                    for k in range(JWAVE):
                        nc.tensor.matmul(
                            accs[k][:, :C],
                            w2_tiles[j][:, k * P : (k + 1) * P],
                            act_tiles[j][:, :C],
                            start=(j == 0),
                            stop=(j == NJ - 1),
                        )
                        if j == NJ - 1:
                            writeback(accs[k], k)
                for hh in range(JWAVE, HC):
                    acc = psD.tile([P, CMAX], F32, tag=f"acc_{hh % JWAVE}", name=f"acc_{s}_{hh}")
                    for j in range(NJ):
                        nc.tensor.matmul(
                            acc[:, :C],
                            w2_tiles[j][:, hh * P : (hh + 1) * P],
                            act_tiles[j][:, :C],
                            start=(j == 0),
                            stop=(j == NJ - 1),
                        )
                    writeback(acc, hh, split=(s == N_SLOTS - 1 and hh == HC - 1))
    nc.compile()
    return nc


def _route(router_logits: np.ndarray, top_k: int):
    """Match jax.nn.softmax + jax.lax.top_k + renormalize (ties -> lower idx)."""
    p = router_logits.astype(np.float64)
    p = np.exp(p - p.max(axis=-1, keepdims=True))
    p /= p.sum(axis=-1, keepdims=True)
    order = np.argsort(-p, axis=-1, kind="stable")
    idx = order[:, :top_k]
    w = np.take_along_axis(p, idx, axis=-1)
    w /= w.sum(axis=-1, keepdims=True)
    return idx, w


def kernel(hidden_states, router_logits, W13, W2, top_k):
    global last_results
    top_k = int(top_k)
    hs = np.asarray(hidden_states, dtype=np.float32)
    T = hs.shape[0]
    idx, w = _route(np.asarray(router_logits, dtype=np.float32), top_k)

    tok_ids, tok_w = [], []
    for e in range(E):
        sel = idx == e  # [T, k]; at most one True per row
        rows = np.nonzero(sel.any(axis=-1))[0]
        tok_ids.append(rows)
        tok_w.append(w[sel].astype(np.float32))  # row-major -> token order

    counts = np.array([len(r) for r in tok_ids])
    order = np.argsort(-counts, kind="stable")
    groups = [order[:4], order[4:]]  # heavy experts in slot 0, light in slot 1

    def pad16(n):
        return max(16, -(-n // 16) * 16)

    caps = [pad16(int(counts[g].max())) for g in groups]
    assert caps[0] <= 512, "column capacity exceeds one PSUM bank"

    W13 = np.asarray(W13, dtype=np.float32)
    W2 = np.asarray(W2, dtype=np.float32)

    in_maps = [dict() for _ in range(N_CORES)]
    for c in range(N_CORES):
        for s in range(N_SLOTS):
            e = int(groups[s][c // 2])
            h = c % 2  # which half of the expert's I channels
            C = caps[s]
            rows = tok_ids[e]
            n = len(rows)
            xt = np.zeros((P, KO, C), dtype=ml_dtypes.bfloat16)
            if n:
                xg = hs[rows].astype(ml_dtypes.bfloat16)  # [n, H]
                xt[:, :, :n] = xg.T.reshape(KO, P, n).transpose(1, 0, 2)
            gate = W13[e][h * IH : (h + 1) * IH]                    # [IH, H]
            up = W13[e][INTER + h * IH : INTER + (h + 1) * IH]     # [IH, H]
            blk = np.concatenate(
                [gate.reshape(NJ, P, H), up.reshape(NJ, P, H)], axis=1
            )  # [NJ, 2P(i), H]
            w13 = np.ascontiguousarray(
                blk.reshape(NJ, 2 * P, KO, P).transpose(0, 3, 2, 1)
            ).astype(ml_dtypes.bfloat16)  # [NJ, P(h), KO, 2P(i)]
            cols = W2[e][:, h * IH : (h + 1) * IH]  # [H, IH]
            w2 = np.ascontiguousarray(cols.T.reshape(NJ, P, H)).astype(
                ml_dtypes.bfloat16
            )  # [NJ, P(i), H]
            in_maps[c][f"xt{s}"] = xt
            in_maps[c][f"w13{s}"] = w13
            in_maps[c][f"w2{s}"] = w2

    nc = _build_nc(caps)
    res = run_bass_kernel_spmd(
        nc,
        in_maps,
        list(range(N_CORES)),
        trace=bool(os.environ.get("MOE_TRACE")),
        tmpdir=os.environ.get("MOE_TRACE_DIR") or None,
    )
    last_results = res

    out = np.zeros((T, H), dtype=np.float32)
    for s in range(N_SLOTS):
        for k in range(len(groups[s])):
            e = int(groups[s][k])
            rows = tok_ids[e]
            n = len(rows)
            if not n:
                continue
            y0 = np.asarray(res.results[2 * k][f"y{s}"], dtype=np.float32)
            y1 = np.asarray(res.results[2 * k + 1][f"y{s}"], dtype=np.float32)
            out[rows] += (y0[:, :n] + y1[:, :n]).T * tok_w[e][:, None]
    return out
